# revision 10
# baseline (speedup 1.0000x reference)
import sys

sys.path.insert(0, "/opt/trn_rl_repo")

import numpy as np
import ml_dtypes

N, F_IN, H, C, E = 20000, 512, 256, 40, 640000
TEMPERATURE = 1.0

NCORE = 8
NT = 157            # number of 128-node tiles (ceil(20000/128)); padded N = 20096
NPAD = NT * 128     # 20096
TPC = 20            # tile-iterations per core (static SPMD loop count)
BASES = [0, 20, 40, 60, 80, 100, 119, 138]
COUNTS = [20, 20, 20, 20, 20, 19, 19, 19]

BF16 = ml_dtypes.bfloat16

_compiled = {}  # CPT -> (nc, sharded_fn, meta)


# ---------------------------------------------------------------- host prep

def _host_prep(x, edge_index, W1, b1, W2, b2):
    """Build per-core index/constant arrays. All heavy math stays on device."""
    src = edge_index[0].astype(np.int64)
    dst = edge_index[1].astype(np.int64)

    deg = np.bincount(dst, minlength=N).astype(np.float32) + 1.0
    dinv = (1.0 / np.sqrt(deg)).astype(np.float32)
    # padded per-node dinv (pad rows get 1.0; their outputs are discarded)
    dinv_pad = np.ones(NPAD, np.float32)
    dinv_pad[:N] = dinv

    # add self edges
    allsrc = np.concatenate([src, np.arange(N, dtype=np.int64)])
    alldst = np.concatenate([dst, np.arange(N, dtype=np.int64)])

    tile_of = alldst // 128
    order = np.argsort(tile_of, kind="stable")
    allsrc, alldst = allsrc[order], alldst[order]
    tcounts = np.bincount(tile_of, minlength=NT)
    CPT = int(np.ceil(tcounts.max() / 128))
    SLOTS = CPT * 128
    tstart = np.zeros(NT + 1, np.int64)
    np.cumsum(tcounts, out=tstart[1:])

    core_of_tile = np.zeros(NT, np.int64)
    for p in range(NCORE):
        core_of_tile[BASES[p]:BASES[p] + COUNTS[p]] = p
    # node -> row in the all-gathered h2 buffer [NCORE*TPC*128, 128]
    t_of_n = np.arange(N) // 128
    h2row = (core_of_tile[t_of_n] * (TPC * 128)
             + (t_of_n - np.array(BASES)[core_of_tile[t_of_n]]) * 128
             + np.arange(N) % 128).astype(np.int64)
    assert h2row.max() < 32768

    def wrap16(idx_slots):  # [T, SLOTS] -> [128, T*SLOTS//16] gather layout
        T = idx_slots.shape[0]
        w = idx_slots.reshape(T, SLOTS // 16, 16)     # slot j = jj*16+pp
        w = np.transpose(w, (2, 0, 1)).reshape(16, T * (SLOTS // 16))
        return np.ascontiguousarray(np.tile(w, (8, 1)).astype(np.int16))

    per_core = []
    for p in range(NCORE):
        g1 = np.zeros((TPC, SLOTS), np.int64)          # src node id (pad -> 0)
        g2 = np.zeros((TPC, SLOTS), np.int64)          # h2 row id  (pad -> 0)
        dstl = np.full((TPC, SLOTS), -1.0, np.float32)  # local dst (pad -> -1)
        for t in range(COUNTS[p]):
            g = BASES[p] + t
            s, e = tstart[g], tstart[g + 1]
            n = e - s
            g1[t, :n] = allsrc[s:e]
            g2[t, :n] = h2row[allsrc[s:e]]
            dstl[t, :n] = (alldst[s:e] - 128 * g).astype(np.float32)
        # dstl layout for DVE: [128 part(e), TPC*CPT] where slot j = c*128 + e
        d = dstl.reshape(TPC, CPT, 128)                 # [t, c, e]
        d = np.transpose(d, (2, 0, 1)).reshape(128, TPC * CPT)
        dl = np.zeros(NPAD, np.float32)
        dl[:N] = dinv
        dloc = np.ones((128, TPC), np.float32)
        for t in range(COUNTS[p]):
            g = BASES[p] + t
            dloc[:, t] = dinv_pad[g * 128:(g + 1) * 128]
        per_core.append(dict(
            gidx1=wrap16(g1),
            gidx2=wrap16(g2),
            dstl=np.ascontiguousarray(d.astype(BF16)),
            dinv_loc=dloc,
        ))

    # shared constants
    xT = np.zeros((4, 128, NPAD), np.float32)
    xT[:, :, :N] = x.T.reshape(4, 128, N)
    shared = dict(
        xT4=xT.astype(BF16),
        W1t=np.ascontiguousarray(W1.reshape(4, 128, H).astype(BF16)),
        W2t=np.ascontiguousarray(W2.reshape(2, 128, C).astype(BF16)),
        b1bc=np.broadcast_to(b1.astype(np.float32), (128, H)).copy(),
        b2bc=np.broadcast_to(b2.astype(np.float32), (128, C)).copy(),
        iota3=np.broadcast_to(np.arange(128, dtype=np.float32), (128, 4, 128)).astype(BF16).copy(),
        ident=np.eye(128, dtype=np.float32).astype(BF16),
        dinv_all=np.ascontiguousarray(dinv_pad.reshape(NT, 128).T.copy()),  # [128, NT]
    )
    in_maps = [{**shared, **pc} for pc in per_core]
    return in_maps, CPT


# ---------------------------------------------------------------- device program

def _build_program(CPT):
    import concourse.bass as bass
    import concourse.tile as tile
    from concourse import bacc, mybir

    SLOTS = CPT * 128
    f32 = mybir.dt.float32
    bf16 = mybir.dt.bfloat16
    i16 = mybir.dt.int16

    nc = bacc.Bacc(None, target_bir_lowering=False, num_devices=NCORE)

    # ---- DRAM I/O
    xT4 = nc.dram_tensor("xT4", [4, 128, NPAD], bf16, kind="ExternalInput")
    W1t = nc.dram_tensor("W1t", [4, 128, H], bf16, kind="ExternalInput")
    W2t = nc.dram_tensor("W2t", [2, 128, C], bf16, kind="ExternalInput")
    b1bc_d = nc.dram_tensor("b1bc", [128, H], f32, kind="ExternalInput")
    b2bc_d = nc.dram_tensor("b2bc", [128, C], f32, kind="ExternalInput")
    iota_d = nc.dram_tensor("iota3", [128, 4, 128], bf16, kind="ExternalInput")
    ident_d = nc.dram_tensor("ident", [128, 128], bf16, kind="ExternalInput")
    dinv_all_d = nc.dram_tensor("dinv_all", [128, NT], f32, kind="ExternalInput")
    dinv_loc_d = nc.dram_tensor("dinv_loc", [128, TPC], f32, kind="ExternalInput")
    dstl_d = nc.dram_tensor("dstl", [128, TPC * CPT], bf16, kind="ExternalInput")
    gidx1_d = nc.dram_tensor("gidx1", [128, TPC * (SLOTS // 16)], i16, kind="ExternalInput")
    gidx2_d = nc.dram_tensor("gidx2", [128, TPC * (SLOTS // 16)], i16, kind="ExternalInput")

    hhat1 = nc.dram_tensor("hhat1", [NPAD, H], bf16)                  # dinv*x@W1, all nodes
    h2loc = nc.dram_tensor("h2loc", [TPC * 128, 128], bf16)           # own dinv*h2, padded cols
    h2full = nc.dram_tensor("h2full", [NCORE * TPC * 128, 128], bf16, addr_space="Shared")
    out_ext = nc.dram_tensor("out", [TPC * 128, C], f32, kind="ExternalOutput")

    with tile.TileContext(nc) as tc:
        with tc.tile_pool(name="const", bufs=1) as cpool:
            W1s = [cpool.tile([128, H], bf16, tag=f"w1_{k}", name=f"W1s{k}") for k in range(4)]
            for k in range(4):
                nc.sync.dma_start(W1s[k][:], W1t[:][k])
            W2s = [cpool.tile([128, C], bf16, tag=f"w2_{k}", name=f"W2s{k}") for k in range(2)]
            for k in range(2):
                nc.sync.dma_start(W2s[k][:], W2t[:][k])
            b1s = cpool.tile([128, H], f32)
            nc.sync.dma_start(b1s[:], b1bc_d[:])
            b2s = cpool.tile([128, C], f32)
            nc.sync.dma_start(b2s[:], b2bc_d[:])
            iota = cpool.tile([128, 4, 128], bf16)
            nc.sync.dma_start(iota[:], iota_d[:])
            ident = cpool.tile([128, 128], bf16)
            nc.sync.dma_start(ident[:], ident_d[:])
            dinv_all = cpool.tile([128, NT], f32)
            nc.sync.dma_start(dinv_all[:], dinv_all_d[:])
            dinv_loc = cpool.tile([128, TPC], f32)
            nc.sync.dma_start(dinv_loc[:], dinv_loc_d[:])
            dstl = cpool.tile([128, TPC * CPT], bf16)
            nc.sync.dma_start(dstl[:], dstl_d[:])
            gidx1 = cpool.tile([128, TPC * (SLOTS // 16)], i16)
            nc.sync.dma_start(gidx1[:], gidx1_d[:])
            gidx2 = cpool.tile([128, TPC * (SLOTS // 16)], i16)
            nc.sync.dma_start(gidx2[:], gidx2_d[:])

            # ---------------- phase B: replicated layer-1 GEMM  hhat1 = dinv * (x @ W1)
            MSLAB = 20  # m-tiles per slab
            with (
                tc.tile_pool(name="xslab", bufs=2) as xpool,
                tc.tile_pool(name="bpsum", bufs=4, space="PSUM") as bpsum,
                tc.tile_pool(name="bev", bufs=4) as bev,
            ):
                for s0 in range(0, NT, MSLAB):
                    mt = min(MSLAB, NT - s0)
                    xs = [xpool.tile([128, mt * 128], bf16, tag=f"xslab{k}", name=f"xs{k}")
                          for k in range(4)]
                    for k in range(4):
                        nc.sync.dma_start(
                            xs[k][:], xT4[:][k][:, s0 * 128:(s0 + mt) * 128])
                    for mi in range(mt):
                        m = s0 + mi
                        ps = bpsum.tile([128, H], f32, tag="bps")
                        for k in range(4):
                            nc.tensor.matmul(
                                ps[:], xs[k][:, mi * 128:(mi + 1) * 128], W1s[k][:],
                                start=(k == 0), stop=(k == 3))
                        hh = bev.tile([128, H], bf16, tag="bhh")
                        nc.scalar.activation(
                            hh[:], ps[:], mybir.ActivationFunctionType.Copy,
                            scale=dinv_all[:, m:m + 1])
                        nc.sync.dma_start(hhat1[m * 128:(m + 1) * 128, :], hh[:])

            # ---------------- phase C: layer-1 aggregation + out1 + h2
            out1T0 = cpool.tile([128, TPC * 128], bf16)   # out1.T rows 0:128
            out1T1 = cpool.tile([128, TPC * 128], bf16)   # out1.T rows 128:256
            h2sb = cpool.tile([128, TPC, 128], bf16)
            nc.vector.memset(h2sb[:], 0.0)

            with (
                tc.tile_pool(name="g1", bufs=2) as gpool,
                tc.tile_pool(name="oh", bufs=6) as ohpool,
                tc.tile_pool(name="cpsum", bufs=2, space="PSUM") as cpsum,
                tc.tile_pool(name="tpsum", bufs=4, space="PSUM") as tpsum,
                tc.tile_pool(name="hpsum", bufs=2, space="PSUM") as hpsum,
                tc.tile_pool(name="cep", bufs=3) as cep,
            ):
                for t in range(TPC):
                    g1 = gpool.tile([128, CPT, H], bf16, tag="g1")
                    nc.gpsimd.dma_gather(
                        out_ap=g1[:], in_ap=hhat1[:],
                        idxs_ap=gidx1[:, t * (SLOTS // 16):(t + 1) * (SLOTS // 16)],
                        num_idxs=SLOTS, num_idxs_reg=SLOTS, elem_size=H,
                        single_packet=False)
                    ps = cpsum.tile([128, H], f32, tag="cps")
                    c = 0
                    while c < CPT:
                        cw = min(4, CPT - c)
                        oh = ohpool.tile([128, 4, 128], bf16, tag="oh")
                        dsl = dstl[:, t * CPT + c: t * CPT + c + cw]
                        nc.vector.tensor_tensor(
                            oh[:, :cw, :],
                            dsl.unsqueeze(2).broadcast_to([128, cw, 128]),
                            iota[:, :cw, :], op=mybir.AluOpType.is_equal)
                        for j in range(cw):
                            nc.tensor.matmul(
                                ps[:], oh[:, j, :], g1[:, c + j, :],
                                start=(c + j == 0), stop=(c + j == CPT - 1))
                        c += cw
                    # epilogue: out1 = relu(dinv*ps + b1)
                    u = cep.tile([128, H], f32, tag="u")
                    nc.scalar.activation(
                        u[:], ps[:], mybir.ActivationFunctionType.Copy,
                        scale=dinv_loc[:, t:t + 1])
                    v = cep.tile([128, H], f32, tag="v")
                    nc.vector.tensor_add(v[:], u[:], b1s[:])
                    o1 = cep.tile([128, H], bf16, tag="o1")
                    nc.vector.tensor_scalar_max(o1[:], v[:], 0.0)
                    # transpose halves into out1T
                    for hh in range(2):
                        tp = tpsum.tile([128, 128], bf16, tag="tp")
                        nc.tensor.transpose(tp[:], o1[:, hh * 128:(hh + 1) * 128], ident[:])
                        dstT = out1T0 if hh == 0 else out1T1
                        nc.vector.tensor_copy(dstT[:, t * 128:(t + 1) * 128], tp[:])
                    # h2 = out1 @ W2, scaled by dinv
                    hp = hpsum.tile([128, C], f32, tag="hp")
                    for k in range(2):
                        dstT = out1T0 if k == 0 else out1T1
                        nc.tensor.matmul(
                            hp[:], dstT[:, t * 128:(t + 1) * 128], W2s[k][:],
                            start=(k == 0), stop=(k == 1))
                    nc.scalar.activation(
                        h2sb[:, t, 0:C], hp[:], mybir.ActivationFunctionType.Copy,
                        scale=dinv_loc[:, t:t + 1])

            nc.sync.dma_start(h2loc[:].rearrange("(t p) c -> p t c", p=128), h2sb[:])

            # ---------------- phase D: all-gather h2
            nc.gpsimd.collective_compute(
                "AllGather", mybir.AluOpType.bypass,
                replica_groups=[list(range(NCORE))],
                ins=[h2loc[:]], outs=[h2full[:]])

            # ---------------- phase E: layer-2 aggregation + output
            with (
                tc.tile_pool(name="g2", bufs=2) as g2pool,
                tc.tile_pool(name="oh2", bufs=6) as oh2pool,
                tc.tile_pool(name="epsum", bufs=2, space="PSUM") as epsum,
                tc.tile_pool(name="eep", bufs=3) as eep,
            ):
                for t in range(TPC):
                    g2 = g2pool.tile([128, CPT, 128], bf16, tag="g2")
                    nc.gpsimd.dma_gather(
                        out_ap=g2[:], in_ap=h2full[:],
                        idxs_ap=gidx2[:, t * (SLOTS // 16):(t + 1) * (SLOTS // 16)],
                        num_idxs=SLOTS, num_idxs_reg=SLOTS, elem_size=128,
                        single_packet=False)
                    ps = epsum.tile([128, C], f32, tag="eps")
                    c = 0
                    while c < CPT:
                        cw = min(4, CPT - c)
                        oh = oh2pool.tile([128, 4, 128], bf16, tag="oh2")
                        dsl = dstl[:, t * CPT + c: t * CPT + c + cw]
                        nc.vector.tensor_tensor(
                            oh[:, :cw, :],
                            dsl.unsqueeze(2).broadcast_to([128, cw, 128]),
                            iota[:, :cw, :], op=mybir.AluOpType.is_equal)
                        for j in range(cw):
                            nc.tensor.matmul(
                                ps[:], oh[:, j, :], g2[:, c + j, 0:C],
                                start=(c + j == 0), stop=(c + j == CPT - 1))
                        c += cw
                    u = eep.tile([128, C], f32, tag="eu")
                    nc.scalar.activation(
                        u[:], ps[:], mybir.ActivationFunctionType.Copy,
                        scale=dinv_loc[:, t:t + 1])
                    o2 = eep.tile([128, C], f32, tag="eo")
                    nc.vector.tensor_add(o2[:], u[:], b2s[:])
                    nc.sync.dma_start(out_ext[t * 128:(t + 1) * 128, :], o2[:])

    nc.compile()
    return nc


# ---------------------------------------------------------------- SPMD runner

def _make_runner(nc):
    import jax
    import numpy as np
    from jax.sharding import Mesh, PartitionSpec, NamedSharding
    from jax.experimental.shard_map import shard_map
    from concourse import bass2jax, mybir

    bass2jax.install_neuronx_cc_hook()

    partition_name = nc.partition_id_tensor.name if nc.partition_id_tensor else None
    in_names, out_names, out_avals, zero_outs = [], [], [], []
    for alloc in nc.m.functions[0].allocations:
        if not isinstance(alloc, mybir.MemoryLocationSet):
            continue
        name = alloc.memorylocations[0].name
        if alloc.kind == "ExternalInput":
            if name != partition_name:
                in_names.append(name)
        elif alloc.kind == "ExternalOutput":
            shape = tuple(alloc.tensor_shape)
            dtype = mybir.dt.np(alloc.dtype)
            out_names.append(name)
            out_avals.append(jax.core.ShapedArray(shape, dtype))
            zero_outs.append(np.zeros(shape, dtype))
    n_params = len(in_names)
    all_in = list(in_names) + list(out_names)
    if partition_name is not None:
        all_in.append(partition_name)

    def _body(*args):
        operands = list(args)
        if partition_name is not None:
            operands.append(bass2jax.partition_id_tensor())
        outs = bass2jax._bass_exec_p.bind(
            *operands,
            out_avals=tuple(out_avals),
            in_names=tuple(all_in),
            out_names=tuple(out_names),
            lowering_input_output_aliases=(),
            sim_require_finite=True,
            sim_require_nnan=True,
            nc=nc,
        )
        return tuple(outs)

    devices = jax.devices()[:NCORE]
    mesh = Mesh(np.asarray(devices), ("core",))
    in_specs = (PartitionSpec("core"),) * (n_params + len(out_names))
    out_specs = (PartitionSpec("core"),) * len(out_names)
    donate = tuple(range(n_params, n_params + len(out_names)))
    sharded = jax.jit(
        shard_map(_body, mesh=mesh, in_specs=in_specs, out_specs=out_specs,
                  check_rep=False),
        donate_argnums=donate, keep_unused=True)

    shard_spec = NamedSharding(mesh, PartitionSpec("core"))

    def run(in_maps, n_timed=0):
        import time
        concat = [np.concatenate([np.asarray(in_maps[c][k]) for c in range(NCORE)], axis=0)
                  for k in in_names]
        dev_in = [jax.device_put(a, shard_spec) for a in concat]
        for a in dev_in:
            a.block_until_ready()

        def one_call():
            zo = [jax.device_put(np.zeros((NCORE * z.shape[0], *z.shape[1:]), z.dtype),
                                 shard_spec) for z in zero_outs]
            outs = sharded(*dev_in, *zo)
            for o in outs:
                o.block_until_ready()
            return outs

        outs = one_call()
        best_ns = None
        for _ in range(n_timed):
            t0 = time.perf_counter()
            outs = one_call()
            dt = (time.perf_counter() - t0) * 1e9
            best_ns = dt if best_ns is None else min(best_ns, dt)
        res = [
            {name: np.asarray(outs[i]).reshape(NCORE, *out_avals[i].shape)[c]
             for i, name in enumerate(out_names)}
            for c in range(NCORE)
        ]
        return res, best_ns

    return run


# ---------------------------------------------------------------- entry point

def _get_compiled(CPT):
    if CPT not in _compiled:
        nc = _build_program(CPT)
        _compiled[CPT] = (nc, _make_runner(nc))
    return _compiled[CPT]


def kernel(x, edge_index, W1, b1, W2, b2, _n_timed=0, _want_time=False):
    x = np.asarray(x, np.float32)
    edge_index = np.asarray(edge_index)
    W1 = np.asarray(W1, np.float32)
    b1 = np.asarray(b1, np.float32)
    W2 = np.asarray(W2, np.float32)
    b2 = np.asarray(b2, np.float32)

    in_maps, CPT = _host_prep(x, edge_index, W1, b1, W2, b2)
    nc, run = _get_compiled(CPT)
    res, best_ns = run(in_maps, n_timed=_n_timed)

    out_full = np.zeros((NPAD, C), np.float32)
    for p in range(NCORE):
        nreal = COUNTS[p] * 128
        lo = BASES[p] * 128
        out_full[lo:lo + nreal] = res[p]["out"][:nreal]
    out = (out_full[:N] / np.float32(TEMPERATURE)).astype(np.float32)
    if _want_time:
        return out, best_ns
    return out
